# revision 15
# baseline (speedup 1.0000x reference)
"""Trainium2 Bass kernel for DSDM cosine-softmin retrieval.

Computes, for a bank A [N, D] and query q [D]:
    sims      = (A @ q) / (||A_r|| * ||q||)           per row r
    weights   = softmax(sims / T)      (== softmin of (1 - sims)/T)
    retrieved = weights @ A                            -> [D]

Sharding: A is split row-wise across 8 NeuronCores (N/8 rows each).
Each core makes a single pass over its shard:
  - DVE: fused multiply+reduce (scalar_tensor_tensor) -> row dots A_r . q
  - ACT: fused Square+accumulate -> row squared norms
  - DVE: Newton rsqrt (3 iters, constant seed ~ 1/2048 since row norms
         concentrate at sqrt(D)) turns (dots, sqnorm) into sims without
         any Ln activation -- keeps ACT on ONE table set (exp_and_others
         holds both Square and Exp), avoiding the ~1.3us table reload
         churn at every group boundary.
  - ACT: w = exp((sim - 1)/T) per group (fixed-shift softmax: sims <= 1
         so exponent <= 0, no max pass -> single pass over A possible)
  - PE : per row-tile matmul with w as the stationary [128,1] operand,
         A as the moving operand accumulating into PSUM. Both operands
         are bitcast to float32r: single-pass fp32 matmul (1 cyc/col at
         moving free dim >= 256) instead of the 4 cyc/col LOW_HIGH
         two-pass fp32 mode that made PE the bottleneck.
Each core then writes its [num (D floats) | den (1 float)] partials to
its output; the 8-way sum and the divide happen on the host as the
unshard step in kernel(). (An on-device AllReduce was measured at
~88us tail even after a warmup collective -- the cores' NEFF launches
are staggered by ~90-120us, so ANY end-of-kernel cross-core sync makes
the earliest core eat the full skew. With per-core partial outputs,
every core's span is just its own DMA-bound work.)

Numerics notes:
  - exp((sim-1)/T) is in [e^-20, 1] for T=0.1 -> fp32 safe without the
    usual running-max correction.
  - The reference's eps clamp max(|a||q|, 1e-8) is a no-op for these
    norms (~sqrt(2048)) and is omitted.
  - Newton for u = rsqrt(nsq), nsq = |a|^2|q|^2: u0 = 1/2048 constant
    (nsq ~ 2048^2 +- ~5%); u' = u(1.5 - 0.5 nsq u^2). 3 iterations give
    rel err < 1e-5 even for 5-sigma rows; weight sensitivity is
    10*|sim|*err ~ 1e-5, far inside the 2e-2 budget.
  - float32r weighted sum ~ tf32-level mantissa; error is random across
    131072 rows and averages out in the softmax-weighted mean.
"""

import sys

import numpy as np

try:
    import concourse.bass as bass
except ImportError:  # fresh grading dir: repo not on sys.path
    sys.path.insert(0, "/opt/trn_rl_repo")
    import concourse.bass as bass

import concourse.bacc as bacc

from contextlib import ExitStack

from concourse import mybir
from concourse.bass_utils import run_bass_kernel_spmd
from concourse.tile import TileContext
from concourse.tile_rust import add_dep_helper

F32 = mybir.dt.float32
F32R = mybir.dt.float32r
BF16 = mybir.dt.bfloat16

N_ADDRESSES = 131072
D = 2048
N_CORES = 8
N_SHARD = N_ADDRESSES // N_CORES  # 16384 rows per core
P = 128                           # SBUF partitions = rows per tile
NT = N_SHARD // P                 # 128 row-tiles per core
CHUNK = 512                       # PE moving free dim (one fp32 PSUM bank)
NCHUNK = D // CHUNK               # 4
TEMPERATURE = 0.1
INV_T = 1.0 / TEMPERATURE

CC_LEN = D + 4  # collective payload: [num(D) | den | pad]

# Epilogue group sizes. Large groups amortize the epilogue; the tapered
# tail keeps the "last tiles can only hit PE after the last DMA" chain
# short.
GROUP_SIZES = [8] * 15 + [4, 2, 1, 1]
assert sum(GROUP_SIZES) == NT
NG = len(GROUP_SIZES)
GMAX = max(GROUP_SIZES)

RSQRT_SEED = 1.0 / 2048.0  # ~ rsqrt(E[|a|^2] * E[|q|^2]) = rsqrt(D*D)
NEWTON_ITERS = 3


def _r(ap: bass.AP) -> bass.AP:
    """Bitcast an f32 AP to float32r for single-pass PE matmul."""
    return ap.bitcast(F32R)


def _build_nc() -> bass.Bass:
    # Bacc (not plain Bass): its finalize() runs generate_event_semaphores,
    # which splits multi-sem waits into EventSemaphore chains -- walrus
    # encodes at most ONE sync wait per compute instruction.
    nc = bacc.Bacc(None, num_devices=N_CORES)

    a_dram = nc.dram_tensor("addresses", [N_SHARD, D], F32, kind="ExternalInput")
    q_dram = nc.dram_tensor("query_address", [1, D], F32, kind="ExternalInput")
    out_dram = nc.dram_tensor("out", [1, CC_LEN], F32, kind="ExternalOutput")

    AF = mybir.ActivationFunctionType
    ALU = mybir.AluOpType

    with ExitStack() as ctx:
        tc = ctx.enter_context(TileContext(nc))
        singles = ctx.enter_context(tc.tile_pool(name="singles", bufs=1))
        a_pool = ctx.enter_context(tc.tile_pool(name="a_pool", bufs=GMAX + 10))
        tmp_pool = ctx.enter_context(tc.tile_pool(name="tmp_pool", bufs=2))
        sq_pool = ctx.enter_context(tc.tile_pool(name="sq_pool", bufs=2))
        stats = ctx.enter_context(tc.tile_pool(name="stats", bufs=4))
        psum = ctx.enter_context(tc.tile_pool(name="psum", bufs=1, space="PSUM"))
        dram = ctx.enter_context(tc.tile_pool(name="dram", bufs=1, space="DRAM"))

        # ---- one-time setup -------------------------------------------------
        # Broadcast q to all 128 partitions. Issue on the gpsimd SWDGE
        # queue: this is a 1 MiB SBUF write, and on the sync HWDGE queue it
        # would serialize ahead of the first A-tile DMAs (~2.7us of start
        # latency on the critical queue).
        q_bcast = singles.tile([P, D], F32)
        q_ap = q_dram[:]
        nc.gpsimd.dma_start(
            out=q_bcast[:],
            in_=bass.AP(tensor=q_ap.tensor, offset=q_ap.offset, ap=[[0, P], q_ap.ap[-1]]),
        )

        # ||q||^2 per partition (identical on all 128). Fused-op `out`
        # operands are mandatory but never read; write them as bf16 to
        # halve the scratch SBUF footprint (accum_out stays f32).
        q_sq_scratch = sq_pool.tile([P, D], BF16, name="stmp_q", tag="stmp")
        qsq = singles.tile([P, 1], F32)
        nc.scalar.activation(
            out=q_sq_scratch[:], in_=q_bcast[:], func=AF.Square, accum_out=qsq[:]
        )
        # qsqm = -0.5 * ||q||^2, folded into the Newton iteration's nsq
        # product: nsqm = sq * qsqm = -0.5 * |a|^2 |q|^2.
        qsqm = singles.tile([P, 1], F32)
        nc.vector.tensor_scalar_mul(qsqm[:], qsq[:], -0.5)

        ones_col = singles.tile([P, 1], F32)
        nc.vector.memset(ones_col[:], 1.0)

        neg_invt = singles.tile([P, 1], F32)
        nc.vector.memset(neg_invt[:], -INV_T)

        den_all = singles.tile([P, NG], F32)

        # PSUM accumulators: weighted-sum chunks (one bank each) + denominator.
        num_psum = [
            psum.tile([1, CHUNK], F32, name=f"num_psum_{c}", tag=f"num_psum_{c}")
            for c in range(NCHUNK)
        ]
        den_psum = psum.tile([1, 1], F32, name="den_psum", tag="den_psum")

        # Scheduler ordering hints: without them, Tile's priority heap places
        # the next group's bulk ops (dots-STT on DVE, Square on ACT) ahead of
        # the previous group's tiny epilogue ops in each engine's stream, so
        # w_g lands late and PE stalls at every group boundary.
        prev_sims = None  # last group's sims TT (DVE)
        prev_w = None     # last group's w Exp (ACT)

        # ---- main pass over row-tiles --------------------------------------
        t_base = 0
        for g, gsz in enumerate(GROUP_SIZES):
            dots_g = stats.tile([P, GMAX], F32, name=f"dots_{g}", tag="dots")
            sq_g = stats.tile([P, GMAX], F32, name=f"sq_{g}", tag="sq")
            a_tiles = []
            for j in range(gsz):
                t = t_base + j
                a_tile = a_pool.tile([P, D], F32, name=f"a_{t}", tag="a")
                # Write through an f32r-typed AP: the BIR verifier requires
                # the producer of an FP32r-matmul operand to be f32r-typed.
                # DMA moves raw bytes either way; PE's f32r mode just reads
                # the fp32 bits with single-pass (reduced-mantissa) math.
                nc.sync.dma_start(
                    out=_r(a_tile[:]), in_=a_dram[t * P : (t + 1) * P, :].bitcast(F32R)
                )
                a_tiles.append(a_tile)

                # dots[r] = sum_d A[r,d] * q[d]   (DVE, fused multiply+reduce)
                ttmp = tmp_pool.tile([P, D], BF16, name=f"ttmp_{t}", tag="ttmp")
                stt_i = nc.vector.scalar_tensor_tensor(
                    out=ttmp[:],
                    in0=a_tile[:],
                    scalar=1.0,
                    in1=q_bcast[:],
                    op0=ALU.mult,
                    op1=ALU.mult,
                    accum_out=dots_g[:, j : j + 1],
                )
                # stt depends on prev group's sims -> sims is placed ahead
                # of the next group's bulk dots in the DVE stream. Skip in
                # the taper: there the hint serializes each tiny group's
                # Newton chain against the next group's dots, adding ~4us
                # of pure-DVE tail per group after the last DMA has landed.
                if prev_sims is not None and gsz == GMAX:
                    add_dep_helper(stt_i.ins, prev_sims.ins, sync=False,
                                   reason="epilogue sims before next dots")
                # sqnorm[r] = sum_d A[r,d]^2      (ACT, fused)
                stmp = sq_pool.tile([P, D], BF16, name=f"stmp_{t}", tag="stmp")
                sq_i = nc.scalar.activation(
                    out=stmp[:],
                    in_=a_tile[:],
                    func=AF.Square,
                    accum_out=sq_g[:, j : j + 1],
                )
                if prev_w is not None:
                    add_dep_helper(sq_i.ins, prev_w.ins, sync=False,
                                   reason="epilogue w before next squares")

            # ---- group epilogue: u = rsqrt(|a|^2 |q|^2) via Newton ---------
            # nsqm = -0.5 * sq * qsq  (one fused tensor_scalar)
            nsqm_g = stats.tile([P, GMAX], F32, name=f"nsqm_{g}", tag="nsqm")
            nc.vector.tensor_scalar(
                nsqm_g[:, :gsz], sq_g[:, :gsz], qsqm[:], None, op0=ALU.mult
            )
            u = stats.tile([P, GMAX], F32, name=f"u_{g}_0", tag="u0")
            nc.vector.memset(u[:], RSQRT_SEED)
            for it in range(NEWTON_ITERS):
                usq = stats.tile([P, GMAX], F32, name=f"usq_{g}_{it}", tag=f"usq{it}")
                nc.vector.tensor_mul(usq[:, :gsz], u[:, :gsz], u[:, :gsz])
                husq = stats.tile([P, GMAX], F32, name=f"husq_{g}_{it}", tag=f"husq{it}")
                nc.vector.tensor_mul(husq[:, :gsz], usq[:, :gsz], nsqm_g[:, :gsz])
                # u' = (husq + 1.5) * u   (one fused scalar_tensor_tensor)
                u_new = stats.tile([P, GMAX], F32, name=f"u_{g}_{it + 1}", tag=f"u{it + 1}")
                nc.vector.scalar_tensor_tensor(
                    out=u_new[:, :gsz],
                    in0=husq[:, :gsz],
                    scalar=1.5,
                    in1=u[:, :gsz],
                    op0=ALU.add,
                    op1=ALU.mult,
                )
                u = u_new

            sims_g = stats.tile([P, GMAX], F32, name=f"sims_{g}", tag="sims")
            prev_sims = nc.vector.tensor_mul(
                sims_g[:, :gsz], dots_g[:, :gsz], u[:, :gsz]
            )
            w_g = stats.tile([P, GMAX], F32, name=f"w_{g}", tag="w")
            prev_w = nc.scalar.activation(
                out=_r(w_g[:, :gsz]),
                in_=sims_g[:, :gsz],
                func=AF.Exp,
                scale=INV_T,
                bias=neg_invt[:],
                accum_out=den_all[:, g : g + 1],
            )

            # ---- weighted sum: PE matmuls, float32r single-pass ------------
            # w column stationary [128,1], A moving [128,512] per chunk,
            # accumulating into [1,512] PSUM banks. PSUM start/stop are
            # bank-scoped; each bank gets start on its first matmul and
            # stop on the last tile's.
            for j in range(gsz):
                t = t_base + j
                for c in range(NCHUNK):
                    nc.tensor.matmul(
                        num_psum[c][:, :],
                        lhsT=_r(w_g[:, j : j + 1]),
                        rhs=_r(a_tiles[j][:, c * CHUNK : (c + 1) * CHUNK]),
                        start=(t == 0),
                        stop=(t == NT - 1),
                    )
            t_base += gsz

        # ---- finalize: write [num | den] partials; host sums and divides ---
        den_col = singles.tile([P, 1], F32)
        nc.vector.reduce_sum(den_col[:], den_all[:], axis=mybir.AxisListType.X)
        nc.tensor.matmul(
            den_psum[:, :], lhsT=ones_col[:], rhs=den_col[:], start=True, stop=True
        )

        final_sb = singles.tile([1, CC_LEN], F32)
        nc.vector.memset(final_sb[:], 0.0)
        for c in range(NCHUNK):
            nc.vector.tensor_copy(
                out=final_sb[0:1, c * CHUNK : (c + 1) * CHUNK], in_=num_psum[c][:, :]
            )
        nc.vector.tensor_copy(out=final_sb[0:1, D : D + 1], in_=den_psum[:, :])
        nc.sync.dma_start(out=out_dram[:], in_=final_sb[:])

    return nc


_NC_CACHE: bass.Bass | None = None


def _get_nc() -> bass.Bass:
    global _NC_CACHE
    if _NC_CACHE is None:
        nc = _build_nc()
        if not nc.is_finalized():
            nc.finalize()  # Bacc: runs the wait-splitting/reg-alloc passes
        _NC_CACHE = nc
    return _NC_CACHE


def run(inputs: dict, **run_kwargs):
    """Run the SPMD kernel; returns (output [D] np.float32, BassKernelResults)."""
    addresses = np.asarray(inputs["addresses"], dtype=np.float32)
    query = np.asarray(inputs["query_address"], dtype=np.float32)
    assert addresses.shape == (N_ADDRESSES, D), addresses.shape
    assert query.shape == (D,), query.shape

    q2d = np.ascontiguousarray(query.reshape(1, D))
    in_maps = [
        {
            "addresses": np.ascontiguousarray(
                addresses[i * N_SHARD : (i + 1) * N_SHARD]
            ),
            "query_address": q2d,
        }
        for i in range(N_CORES)
    ]
    res = run_bass_kernel_spmd(_get_nc(), in_maps, list(range(N_CORES)), **run_kwargs)
    # Unshard: each core returns [num (D) | den | pad] partials over its
    # row shard; sum across cores and normalize.
    parts = np.stack(
        [np.asarray(res.results[i]["out"], dtype=np.float32).reshape(CC_LEN)
         for i in range(N_CORES)]
    ).astype(np.float64)
    tot = parts.sum(axis=0)
    out = (tot[:D] / tot[D]).astype(np.float32)
    return out, res


def kernel(**inputs) -> np.ndarray:
    out, _ = run(inputs)
    return out


# revision 20
# speedup vs baseline: 1.0696x; 1.0696x over previous
"""Trainium2 Bass kernel for DSDM cosine-softmin retrieval.

Computes, for a bank A [N, D] and query q [D]:
    sims      = (A @ q) / (||A_r|| * ||q||)           per row r
    weights   = softmax(sims / T)      (== softmin of (1 - sims)/T)
    retrieved = weights @ A                            -> [D]

Sharding: A is split row-wise across 8 NeuronCores (N/8 rows each).
Each core makes a single pass over its shard:
  - DVE: fused multiply+reduce (scalar_tensor_tensor) -> row dots A_r . q
  - ACT: fused Square+accumulate -> row squared norms
  - DVE: Newton rsqrt (3 iters, constant seed ~ 1/2048 since row norms
         concentrate at sqrt(D)) turns (dots, sqnorm) into sims without
         any Ln activation -- keeps ACT on ONE table set (exp_and_others
         holds both Square and Exp), avoiding the ~1.3us table reload
         churn at every group boundary.
  - ACT: w = exp((sim - 1)/T) per group (fixed-shift softmax: sims <= 1
         so exponent <= 0, no max pass -> single pass over A possible)
  - PE : per row-tile matmul with w as the stationary [128,1] operand,
         A as the moving operand accumulating into PSUM. Both operands
         are bitcast to float32r: single-pass fp32 matmul (1 cyc/col at
         moving free dim >= 256) instead of the 4 cyc/col LOW_HIGH
         two-pass fp32 mode that made PE the bottleneck.
Each core then writes its [num (D floats) | den (1 float)] partials to
its output; the 8-way sum and the divide happen on the host as the
unshard step in kernel(). (An on-device AllReduce was measured at
~88us tail even after a warmup collective -- the cores' NEFF launches
are staggered by ~90-120us, so ANY end-of-kernel cross-core sync makes
the earliest core eat the full skew. With per-core partial outputs,
every core's span is just its own DMA-bound work.)

Numerics notes:
  - exp((sim-1)/T) is in [e^-20, 1] for T=0.1 -> fp32 safe without the
    usual running-max correction.
  - The reference's eps clamp max(|a||q|, 1e-8) is a no-op for these
    norms (~sqrt(2048)) and is omitted.
  - Newton for u = rsqrt(nsq), nsq = |a|^2|q|^2: u0 = 1/2048 constant
    (nsq ~ 2048^2 +- ~5%); u' = u(1.5 - 0.5 nsq u^2). 3 iterations give
    rel err < 1e-5 even for 5-sigma rows; weight sensitivity is
    10*|sim|*err ~ 1e-5, far inside the 2e-2 budget.
  - float32r weighted sum ~ tf32-level mantissa; error is random across
    131072 rows and averages out in the softmax-weighted mean.
"""

import sys

import numpy as np

try:
    import concourse.bass as bass
except ImportError:  # fresh grading dir: repo not on sys.path
    sys.path.insert(0, "/opt/trn_rl_repo")
    import concourse.bass as bass

import concourse.bacc as bacc

from contextlib import ExitStack

from concourse import mybir
from concourse.bass_utils import run_bass_kernel_spmd
from concourse.tile import TileContext
from concourse.tile_rust import add_dep_helper

F32 = mybir.dt.float32
F32R = mybir.dt.float32r
BF16 = mybir.dt.bfloat16

N_ADDRESSES = 131072
D = 2048
N_CORES = 8
N_SHARD = N_ADDRESSES // N_CORES  # 16384 rows per core
P = 128                           # SBUF partitions = rows per tile
NT = N_SHARD // P                 # 128 row-tiles per core
CHUNK = 512                       # PE moving free dim (one fp32 PSUM bank)
NCHUNK = D // CHUNK               # 4
TEMPERATURE = 0.1
INV_T = 1.0 / TEMPERATURE

CC_LEN = D + 4  # collective payload: [num(D) | den | pad]

# Epilogue group sizes. Large groups amortize the epilogue; the tapered
# tail keeps the "last tiles can only hit PE after the last DMA" chain
# short.
GROUP_SIZES = [8] * 15 + [4, 2, 1, 1]
assert sum(GROUP_SIZES) == NT
NG = len(GROUP_SIZES)
GMAX = max(GROUP_SIZES)

RSQRT_SEED = 1.0 / 2048.0  # ~ rsqrt(E[|a|^2] * E[|q|^2]) = rsqrt(D*D)
NEWTON_ITERS = 3


def _r(ap: bass.AP) -> bass.AP:
    """Bitcast an f32 AP to float32r for single-pass PE matmul."""
    return ap.bitcast(F32R)


def _build_nc() -> bass.Bass:
    # Bacc (not plain Bass): its finalize() runs generate_event_semaphores,
    # which splits multi-sem waits into EventSemaphore chains -- walrus
    # encodes at most ONE sync wait per compute instruction.
    nc = bacc.Bacc(None, num_devices=N_CORES)

    a_dram = nc.dram_tensor("addresses", [N_SHARD, D], F32, kind="ExternalInput")
    q_dram = nc.dram_tensor("query_address", [1, D], F32, kind="ExternalInput")
    out_dram = nc.dram_tensor("out", [1, CC_LEN], F32, kind="ExternalOutput")

    AF = mybir.ActivationFunctionType
    ALU = mybir.AluOpType

    with ExitStack() as ctx:
        tc = ctx.enter_context(TileContext(nc))
        singles = ctx.enter_context(tc.tile_pool(name="singles", bufs=1))
        # 9 bufs x [128, 2D] pair tiles = 18 row-tiles in flight (144 KiB
        # per partition).
        a_pool = ctx.enter_context(tc.tile_pool(name="a_pool", bufs=9))
        tmp_pool = ctx.enter_context(tc.tile_pool(name="tmp_pool", bufs=2))
        sq_pool = ctx.enter_context(tc.tile_pool(name="sq_pool", bufs=2))
        stats = ctx.enter_context(tc.tile_pool(name="stats", bufs=4))
        psum = ctx.enter_context(tc.tile_pool(name="psum", bufs=1, space="PSUM"))
        dram = ctx.enter_context(tc.tile_pool(name="dram", bufs=1, space="DRAM"))

        # ---- one-time setup -------------------------------------------------
        # Broadcast q to all 128 partitions. (Keep this on the sync HWDGE
        # queue: routing it via gpsimd SWDGE was measured to dribble 128
        # chunks through the shared SDMA engines for ~70us, dropping the
        # A-stream from 384 to 353 GB/s.)
        q_bcast = singles.tile([P, D], F32)
        q_ap = q_dram[:]
        nc.sync.dma_start(
            out=q_bcast[:],
            in_=bass.AP(tensor=q_ap.tensor, offset=q_ap.offset, ap=[[0, P], q_ap.ap[-1]]),
        )

        # ||q||^2 per partition (identical on all 128). Fused-op `out`
        # operands are mandatory but never read; write them as bf16 to
        # halve the scratch SBUF footprint (accum_out stays f32).
        q_sq_scratch = sq_pool.tile([P, D], BF16, name="stmp_q", tag="stmp")
        qsq = singles.tile([P, 1], F32)
        nc.scalar.activation(
            out=q_sq_scratch[:], in_=q_bcast[:], func=AF.Square, accum_out=qsq[:]
        )
        # qsqm = -0.5 * ||q||^2, folded into the Newton iteration's nsq
        # product: nsqm = sq * qsqm = -0.5 * |a|^2 |q|^2.
        qsqm = singles.tile([P, 1], F32)
        nc.vector.tensor_scalar_mul(qsqm[:], qsq[:], -0.5)

        ones_col = singles.tile([P, 1], F32)
        nc.vector.memset(ones_col[:], 1.0)

        neg_invt = singles.tile([P, 1], F32)
        nc.vector.memset(neg_invt[:], -INV_T)

        den_all = singles.tile([P, NG], F32)

        # PSUM accumulators: weighted-sum chunks (one bank each) + denominator.
        num_psum = [
            psum.tile([1, CHUNK], F32, name=f"num_psum_{c}", tag=f"num_psum_{c}")
            for c in range(NCHUNK)
        ]
        den_psum = psum.tile([1, 1], F32, name="den_psum", tag="den_psum")

        # Scheduler ordering hints: without them, Tile's priority heap places
        # the next group's bulk ops (dots-STT on DVE, Square on ACT) ahead of
        # the previous group's tiny epilogue ops in each engine's stream, so
        # w_g lands late and PE stalls at every group boundary.
        prev_sims = None  # last group's sims TT (DVE)
        prev_w = None     # last group's w Exp (ACT)

        # ---- main pass over row-tiles --------------------------------------
        # A-tile DMAs move TWO row-tiles per dma_start ([128, 2D], 16 KiB
        # per partition, src rows r and r+128 side by side): halves the
        # instruction + completion-handshake count on the DMA ring.
        a_base = a_dram[:]
        pair_views: dict[int, bass.AP] = {}

        def a_view(t: int) -> bass.AP:
            if t not in pair_views:
                base = t - (t % 2)
                pt = a_pool.tile([P, 2 * D], F32, name=f"apair_{base}", tag="a")
                src = bass.AP(
                    tensor=a_base.tensor,
                    offset=base * P * D,
                    ap=[[D, P], [P * D, 2], [1, D]],
                )
                # Write through f32r-typed APs: the BIR verifier requires
                # the producer of an FP32r-matmul operand to be f32r-typed.
                # DMA moves raw bytes either way; PE's f32r mode just reads
                # the fp32 bits with single-pass (reduced-mantissa) math.
                nc.sync.dma_start(out=_r(pt[:]), in_=src.bitcast(F32R))
                pair_views[base] = pt[:, 0:D]
                pair_views[base + 1] = pt[:, D : 2 * D]
            return pair_views[t]

        t_base = 0
        for g, gsz in enumerate(GROUP_SIZES):
            dots_g = stats.tile([P, GMAX], F32, name=f"dots_{g}", tag="dots")
            sq_g = stats.tile([P, GMAX], F32, name=f"sq_{g}", tag="sq")
            a_tiles = []
            for j in range(gsz):
                t = t_base + j
                a_tile = a_view(t)
                a_tiles.append(a_tile)

                # dots[r] = sum_d A[r,d] * q[d]   (DVE, fused multiply+reduce)
                ttmp = tmp_pool.tile([P, D], BF16, name=f"ttmp_{t}", tag="ttmp")
                stt_i = nc.vector.scalar_tensor_tensor(
                    out=ttmp[:],
                    in0=a_tile[:],
                    scalar=1.0,
                    in1=q_bcast[:],
                    op0=ALU.mult,
                    op1=ALU.mult,
                    accum_out=dots_g[:, j : j + 1],
                )
                # stt depends on prev group's sims -> sims is placed ahead
                # of the next group's bulk dots in the DVE stream.
                if prev_sims is not None:
                    add_dep_helper(stt_i.ins, prev_sims.ins, sync=False,
                                   reason="epilogue sims before next dots")
                # sqnorm[r] = sum_d A[r,d]^2      (ACT, fused)
                stmp = sq_pool.tile([P, D], BF16, name=f"stmp_{t}", tag="stmp")
                sq_i = nc.scalar.activation(
                    out=stmp[:],
                    in_=a_tile[:],
                    func=AF.Square,
                    accum_out=sq_g[:, j : j + 1],
                )
                if prev_w is not None:
                    add_dep_helper(sq_i.ins, prev_w.ins, sync=False,
                                   reason="epilogue w before next squares")

            # ---- group epilogue: u = rsqrt(|a|^2 |q|^2) via Newton ---------
            # nsqm = -0.5 * sq * qsq  (one fused tensor_scalar)
            nsqm_g = stats.tile([P, GMAX], F32, name=f"nsqm_{g}", tag="nsqm")
            nc.vector.tensor_scalar(
                nsqm_g[:, :gsz], sq_g[:, :gsz], qsqm[:], None, op0=ALU.mult
            )
            u = stats.tile([P, GMAX], F32, name=f"u_{g}_0", tag="u0")
            nc.vector.memset(u[:], RSQRT_SEED)
            for it in range(NEWTON_ITERS):
                usq = stats.tile([P, GMAX], F32, name=f"usq_{g}_{it}", tag=f"usq{it}")
                nc.vector.tensor_mul(usq[:, :gsz], u[:, :gsz], u[:, :gsz])
                husq = stats.tile([P, GMAX], F32, name=f"husq_{g}_{it}", tag=f"husq{it}")
                nc.vector.tensor_mul(husq[:, :gsz], usq[:, :gsz], nsqm_g[:, :gsz])
                # u' = (husq + 1.5) * u   (one fused scalar_tensor_tensor)
                u_new = stats.tile([P, GMAX], F32, name=f"u_{g}_{it + 1}", tag=f"u{it + 1}")
                nc.vector.scalar_tensor_tensor(
                    out=u_new[:, :gsz],
                    in0=husq[:, :gsz],
                    scalar=1.5,
                    in1=u[:, :gsz],
                    op0=ALU.add,
                    op1=ALU.mult,
                )
                u = u_new

            sims_g = stats.tile([P, GMAX], F32, name=f"sims_{g}", tag="sims")
            prev_sims = nc.vector.tensor_mul(
                sims_g[:, :gsz], dots_g[:, :gsz], u[:, :gsz]
            )
            w_g = stats.tile([P, GMAX], F32, name=f"w_{g}", tag="w")
            prev_w = nc.scalar.activation(
                out=_r(w_g[:, :gsz]),
                in_=sims_g[:, :gsz],
                func=AF.Exp,
                scale=INV_T,
                bias=neg_invt[:],
                accum_out=den_all[:, g : g + 1],
            )

            # ---- weighted sum: PE matmuls, float32r single-pass ------------
            # w column stationary [128,1], A moving [128,512] per chunk,
            # accumulating into [1,512] PSUM banks. PSUM start/stop are
            # bank-scoped; each bank gets start on its first matmul and
            # stop on the last tile's.
            for j in range(gsz):
                t = t_base + j
                for c in range(NCHUNK):
                    nc.tensor.matmul(
                        num_psum[c][:, :],
                        lhsT=_r(w_g[:, j : j + 1]),
                        rhs=_r(a_tiles[j][:, c * CHUNK : (c + 1) * CHUNK]),
                        start=(t == 0),
                        stop=(t == NT - 1),
                    )
            t_base += gsz

        # ---- finalize: write [num | den] partials; host sums and divides ---
        den_col = singles.tile([P, 1], F32)
        nc.vector.reduce_sum(den_col[:], den_all[:], axis=mybir.AxisListType.X)
        nc.tensor.matmul(
            den_psum[:, :], lhsT=ones_col[:], rhs=den_col[:], start=True, stop=True
        )

        final_sb = singles.tile([1, CC_LEN], F32)
        nc.vector.memset(final_sb[:], 0.0)
        for c in range(NCHUNK):
            nc.vector.tensor_copy(
                out=final_sb[0:1, c * CHUNK : (c + 1) * CHUNK], in_=num_psum[c][:, :]
            )
        nc.vector.tensor_copy(out=final_sb[0:1, D : D + 1], in_=den_psum[:, :])
        nc.sync.dma_start(out=out_dram[:], in_=final_sb[:])

    return nc


_NC_CACHE: bass.Bass | None = None


def _get_nc() -> bass.Bass:
    global _NC_CACHE
    if _NC_CACHE is None:
        nc = _build_nc()
        if not nc.is_finalized():
            nc.finalize()  # Bacc: runs the wait-splitting/reg-alloc passes
        _NC_CACHE = nc
    return _NC_CACHE


def run(inputs: dict, **run_kwargs):
    """Run the SPMD kernel; returns (output [D] np.float32, BassKernelResults)."""
    addresses = np.asarray(inputs["addresses"], dtype=np.float32)
    query = np.asarray(inputs["query_address"], dtype=np.float32)
    assert addresses.shape == (N_ADDRESSES, D), addresses.shape
    assert query.shape == (D,), query.shape

    q2d = np.ascontiguousarray(query.reshape(1, D))
    in_maps = [
        {
            "addresses": np.ascontiguousarray(
                addresses[i * N_SHARD : (i + 1) * N_SHARD]
            ),
            "query_address": q2d,
        }
        for i in range(N_CORES)
    ]
    res = run_bass_kernel_spmd(_get_nc(), in_maps, list(range(N_CORES)), **run_kwargs)
    # Unshard: each core returns [num (D) | den | pad] partials over its
    # row shard; sum across cores and normalize.
    parts = np.stack(
        [np.asarray(res.results[i]["out"], dtype=np.float32).reshape(CC_LEN)
         for i in range(N_CORES)]
    ).astype(np.float64)
    tot = parts.sum(axis=0)
    out = (tot[:D] / tot[D]).astype(np.float32)
    return out, res


def kernel(**inputs) -> np.ndarray:
    out, _ = run(inputs)
    return out
